# revision 7
# baseline (speedup 1.0000x reference)
"""CrossAttention Trainium2 kernel (8 NeuronCores, SPMD).

Problem: x [4,256,64,64], context [4,512,32,32], 8 heads x 64 dim,
q = Wq@x, k = Wk@ctx, v = Wv@ctx, attn = softmax(q^T k / 8), out = Wo@(v attn^T) + bo.

Sharding: fully data-parallel over (batch, query-spatial-half) -> 8 shards.
Each core computes K/V for its batch (duplicated per pair) and attention +
output projection for its 2048 query positions. Zero collectives.

Per-core pipeline (all matmuls in float32r = full-rate tf32-like):
  simT[j,i] = k^T q per head (j on partitions -> softmax denominator via a
  ones-column appended to V in the AV matmul), exp on ScalarE reading PSUM,
  AV accumulation with M=65 (64 v-channels + colsum row), normalization via
  reciprocal + DRAM-bounce partition-broadcast + DVE multiply, output
  projection y = Wo @ avn with per-partition bias add.
"""
import os
import sys
import numpy as np

for _p in ("/opt/trn_rl_repo", "/root/.axon_site/_ro/trn_rl_repo"):
    if os.path.isdir(_p) and _p not in sys.path:
        sys.path.insert(0, _p)

import concourse.bass as bass
import concourse.mybir as mybir
from concourse.tile import TileContext
from concourse.bass_utils import run_bass_kernel_spmd

F32 = mybir.dt.float32
F32R = mybir.dt.float32r
EXP = mybir.ActivationFunctionType.Exp

B, H, D = 4, 8, 64
EQ, EK = 256, 512          # x channels, ctx channels
NQ, NK = 2048, 1024        # per-core query positions, kv positions
OC = 256                   # output channels
SCALE = D ** -0.5
IT, JT = NQ // 512, NK // 128   # 4 i-tiles of 512, 8 j-tiles of 128


def _split_drain_waits(nc, max_waits=1):
    """This walrus build rejects Drain instructions with >max_waits sem
    waits; move the extra waits onto standalone nops just before."""
    n_new = 0
    for f in nc.m.functions:
        for bb in f.blocks:
            insts = list(bb.instructions)
            out = []
            changed = False
            for inst in insts:
                si = inst.sync_info
                if si is not None and si.on_wait and len(si.on_wait) > max_waits:
                    waits = list(si.on_wait)
                    for w in waits[:-max_waits]:
                        nop = mybir.InstNoOp(
                            name=f"I-splitw-{n_new}",
                            sync_info=mybir.SyncInfo(on_wait=[w], on_update=[]),
                        )
                        nop.engine = inst.engine
                        n_new += 1
                        out.append(nop)
                        nc.register_instruction(nop, overwrite=True)
                    si.on_wait = waits[-max_waits:]
                    inst.sync_info = si
                    changed = True
                out.append(inst)
            if changed:
                bb.instructions.clear()
                bb.instructions.extend(out)
    return n_new


def _build():
    nc = bass.Bass()
    x_s = nc.declare_dram_parameter("x_s", [EQ, NQ], F32R, isOutput=False)
    ctx_s = nc.declare_dram_parameter("ctx_s", [EK, NK], F32R, isOutput=False)
    WqT = nc.declare_dram_parameter("WqT", [EQ, 512], F32R, isOutput=False)
    WkT = nc.declare_dram_parameter("WkT", [EK, 512], F32R, isOutput=False)
    WvT = nc.declare_dram_parameter("WvT", [EK, 512], F32R, isOutput=False)
    WoT = nc.declare_dram_parameter("WoT", [512, OC], F32R, isOutput=False)
    bo = nc.declare_dram_parameter("bo", [OC], F32, isOutput=False)
    y = nc.declare_dram_parameter("y", [OC, NQ], F32, isOutput=True)

    sscratch = nc.dram_tensor("sscratch", [IT, 4, 2, 512], F32)

    with TileContext(nc) as tc:
        with (
            tc.tile_pool(name="consts", bufs=1) as cp,
            tc.tile_pool(name="qkv", bufs=1) as qp,
            tc.tile_pool(name="exps", bufs=3) as ep,
            tc.tile_pool(name="avnp", bufs=6) as avnp,
            tc.tile_pool(name="work", bufs=4) as wp,
        ):
            # ---- const / weight / input loads ----
            wqt = cp.tile([128, 2 * 512], F32R, tag="wqt")
            wkt = cp.tile([128, 4 * 512], F32R, tag="wkt")
            wvt = cp.tile([128, 4 * 512], F32R, tag="wvt")
            wot = cp.tile([128, 4 * OC], F32R, tag="wot")
            bo_col = cp.tile([128, 2], F32, tag="bo_col")
            x_sb = cp.tile([128, 2 * NQ], F32R, tag="x_sb")
            ctx_sb = cp.tile([128, 4 * NK], F32R, tag="ctx_sb")
            for ec in range(2):
                nc.sync.dma_start(out=wqt[:, ec * 512:(ec + 1) * 512],
                                  in_=WqT[ec * 128:(ec + 1) * 128, :])
                nc.sync.dma_start(out=x_sb[:, ec * NQ:(ec + 1) * NQ],
                                  in_=x_s[ec * 128:(ec + 1) * 128, :])
            for ec in range(4):
                nc.sync.dma_start(out=wkt[:, ec * 512:(ec + 1) * 512],
                                  in_=WkT[ec * 128:(ec + 1) * 128, :])
                nc.sync.dma_start(out=wvt[:, ec * 512:(ec + 1) * 512],
                                  in_=WvT[ec * 128:(ec + 1) * 128, :])
                nc.sync.dma_start(out=wot[:, ec * OC:(ec + 1) * OC],
                                  in_=WoT[ec * 128:(ec + 1) * 128, :])
                nc.sync.dma_start(out=ctx_sb[:, ec * NK:(ec + 1) * NK],
                                  in_=ctx_s[ec * 128:(ec + 1) * 128, :])
            for ob in range(2):
                nc.sync.dma_start(out=bo_col[:, ob:ob + 1],
                                  in_=bo[ob * 128:(ob + 1) * 128])

            # persistent activations
            q_sb = qp.tile([128, 4 * NQ], F32R, tag="q_sb")    # [hp, i]
            k_sb = qp.tile([128, 4 * NK], F32R, tag="k_sb")    # [hp, j]
            vt_sb = qp.tile([128, JT * 520], F32R, tag="vt_sb")  # [jt, h*65 + c]

            # ones columns of vt (col 64 of each 65-block)
            vt_4d = vt_sb.rearrange("p (j h c) -> p j h c", j=JT, h=H)
            ones_f32 = cp.tile([128, JT * H], F32, tag="ones_f32")
            nc.vector.memset(ones_f32, 1.0)
            nc.vector.tensor_copy(
                vt_4d[:, :, :, 64:65],
                ones_f32.rearrange("p (j h) -> p j h", j=JT).unsqueeze(-1))

            # ---- projections ----
            with tc.tile_pool(name="pp", bufs=3, space="PSUM") as pp:
                # K: k[c,j] ; c-block hp(128 = 2 heads), contraction 4x128
                for hp in range(4):
                    for ntile in range(NK // 512):
                        pk = pp.tile([128, 512], F32, tag="pk")
                        for ec in range(4):
                            nc.tensor.matmul(
                                pk,
                                lhsT=wkt[:, ec * 512 + hp * 128: ec * 512 + (hp + 1) * 128],
                                rhs=ctx_sb[:, ec * NK + ntile * 512: ec * NK + (ntile + 1) * 512],
                                start=(ec == 0), stop=(ec == 3))
                        nc.vector.tensor_copy(
                            k_sb[:, hp * NK + ntile * 512: hp * NK + (ntile + 1) * 512], pk)
                # VT: vt[j, c] all heads; j-block jt(128), contraction 4x128
                for jt in range(JT):
                    pv = pp.tile([128, 512], F32, tag="pk")
                    for ec in range(4):
                        nc.tensor.matmul(
                            pv,
                            lhsT=ctx_sb[:, ec * NK + jt * 128: ec * NK + (jt + 1) * 128],
                            rhs=wvt[:, ec * 512:(ec + 1) * 512],
                            start=(ec == 0), stop=(ec == 3))
                    vt_t = vt_sb[:, jt * 520:(jt + 1) * 520].rearrange(
                        "p (h c) -> p h c", h=H)[:, :, 0:64]
                    nc.vector.tensor_copy(vt_t, pv.rearrange("p (h c) -> p h c", c=64))
                # Q: q[c,i]; c-block hp, contraction 2x128
                for hp in range(4):
                    for ntile in range(IT):
                        pq = pp.tile([128, 512], F32, tag="pk")
                        for ec in range(2):
                            nc.tensor.matmul(
                                pq,
                                lhsT=wqt[:, ec * 512 + hp * 128: ec * 512 + (hp + 1) * 128],
                                rhs=x_sb[:, ec * NQ + ntile * 512: ec * NQ + (ntile + 1) * 512],
                                start=(ec == 0), stop=(ec == 1))
                        nc.vector.tensor_copy(
                            q_sb[:, hp * NQ + ntile * 512: hp * NQ + (ntile + 1) * 512], pq)

            # ---- attention + output projection ----
            with (
                tc.tile_pool(name="slab", bufs=2, space="PSUM") as slabp,
                tc.tile_pool(name="avp", bufs=1, space="PSUM") as avp,
                tc.tile_pool(name="yp", bufs=2, space="PSUM") as yp,
            ):
                for it in range(IT):
                    avn_tiles = []
                    for hp in range(4):
                        av0 = avp.tile([65, 512], F32, tag="av0")
                        av1 = avp.tile([65, 512], F32, tag="av1")
                        qs = slice(hp * NQ + it * 512, hp * NQ + (it + 1) * 512)
                        for jt in range(JT):
                            slab = slabp.tile([128, 1024], F32, tag="slab")
                            ks = slice(hp * NK + jt * 128, hp * NK + (jt + 1) * 128)
                            # packed pair: head 2hp (rows 0-63), head 2hp+1 (rows 64-127)
                            nc.tensor.matmul(
                                slab[:, 0:512], lhsT=k_sb[0:64, ks],
                                rhs=q_sb[0:64, qs],
                                start=True, stop=True, tile_position=(0, 0))
                            nc.tensor.matmul(
                                slab[:, 512:1024], lhsT=k_sb[64:128, ks],
                                rhs=q_sb[64:128, qs],
                                start=True, stop=True, tile_position=(64, 0))
                            exps = ep.tile([128, 1024], F32R, tag="exps")
                            nc.scalar.activation(exps, slab, EXP,
                                                 bias=0.0, scale=SCALE)
                            nc.tensor.matmul(
                                av0,
                                lhsT=vt_sb[:, jt * 520 + (2 * hp) * 65: jt * 520 + (2 * hp) * 65 + 65],
                                rhs=exps[:, 0:512],
                                start=(jt == 0), stop=(jt == JT - 1))
                            nc.tensor.matmul(
                                av1,
                                lhsT=vt_sb[:, jt * 520 + (2 * hp + 1) * 65: jt * 520 + (2 * hp + 1) * 65 + 65],
                                rhs=exps[:, 512:1024],
                                start=(jt == 0), stop=(jt == JT - 1))
                        # softmax normalization
                        srow_a = wp.tile([1, 512], F32, tag="srow_a")
                        srow_b = wp.tile([1, 512], F32, tag="srow_b")
                        nc.vector.reciprocal(srow_a, av0[64:65, :])
                        nc.vector.reciprocal(srow_b, av1[64:65, :])
                        nc.sync.dma_start(out=sscratch[it, hp, 0], in_=srow_a)
                        nc.sync.dma_start(out=sscratch[it, hp, 1], in_=srow_b)
                        sbc = wp.tile([128, 512], F32, tag="sbc")
                        base = ((it * 4 + hp) * 2) * 512
                        nc.sync.dma_start(
                            out=sbc[0:64, :],
                            in_=bass.AP(tensor=sscratch, offset=base,
                                        ap=[[0, 64], [1, 512]]))
                        nc.sync.dma_start(
                            out=sbc[64:128, :],
                            in_=bass.AP(tensor=sscratch, offset=base + 512,
                                        ap=[[0, 64], [1, 512]]))
                        avn = avnp.tile([128, 512], F32R, tag="avn")
                        nc.vector.tensor_mul(avn[0:64, :], av0[0:64, :], sbc[0:64, :])
                        nc.vector.tensor_mul(avn[64:128, :], av1[0:64, :], sbc[64:128, :])
                        avn_tiles.append(avn)
                    # output projection for this i-tile
                    for ob in range(2):
                        yps = yp.tile([128, 512], F32, tag="yps")
                        for cc in range(4):
                            nc.tensor.matmul(
                                yps,
                                lhsT=wot[:, cc * OC + ob * 128: cc * OC + (ob + 1) * 128],
                                rhs=avn_tiles[cc],
                                start=(cc == 0), stop=(cc == 3))
                        ysb = wp.tile([128, 512], F32, tag="ysb")
                        nc.vector.tensor_scalar_add(ysb, yps, bo_col[:, ob:ob + 1])
                        nc.sync.dma_start(
                            out=y[ob * 128:(ob + 1) * 128, it * 512:(it + 1) * 512],
                            in_=ysb)

    _split_drain_waits(nc)
    return nc


_CACHED = None


def kernel(x, context, Wq, Wk, Wv, Wo, bo):
    global _CACHED
    if _CACHED is None:
        _CACHED = _build()
    nc = _CACHED

    x = np.asarray(x, dtype=np.float32)
    context = np.asarray(context, dtype=np.float32)
    xf = x.reshape(B, EQ, 64 * 64)
    cf = context.reshape(B, EK, 32 * 32)
    WqT = np.ascontiguousarray(np.asarray(Wq, np.float32).T)
    WkT = np.ascontiguousarray(np.asarray(Wk, np.float32).T)
    WvT = np.ascontiguousarray(np.asarray(Wv, np.float32).T)
    WoT = np.ascontiguousarray(np.asarray(Wo, np.float32).T)
    bo = np.ascontiguousarray(np.asarray(bo, np.float32))

    in_maps = []
    for core in range(8):
        b, half = core // 2, core % 2
        in_maps.append({
            "x_s": np.ascontiguousarray(xf[b, :, half * NQ:(half + 1) * NQ]),
            "ctx_s": np.ascontiguousarray(cf[b]),
            "WqT": WqT, "WkT": WkT, "WvT": WvT, "WoT": WoT, "bo": bo,
        })

    res = run_bass_kernel_spmd(nc, in_maps, list(range(8)))
    kernel.last_results = res

    out = np.empty((B, OC, 64 * 64), dtype=np.float32)
    for core in range(8):
        b, half = core // 2, core % 2
        out[b, :, half * NQ:(half + 1) * NQ] = res.results[core]["y"]
    return out.reshape(B, OC, 64, 64)


# revision 11
# speedup vs baseline: 1.1275x; 1.1275x over previous
"""CrossAttention Trainium2 kernel (8 NeuronCores, SPMD).

Problem: x [4,256,64,64], context [4,512,32,32], 8 heads x 64 dim,
q = Wq@x, k = Wk@ctx, v = Wv@ctx, attn = softmax(q^T k / 8), out = Wo@(v attn^T) + bo.

Sharding: fully data-parallel over (batch, query-spatial-half) -> 8 shards.
Each core computes K/V for its batch (duplicated per pair) and attention +
output projection for its 2048 query positions. Zero collectives.

Per-core pipeline (matmuls in float32r = full-rate):
  simT[j,i] = k^T q per head-pair, two K=64 matmuls packed onto PE row
  groups (tile_position); exp on ScalarE straight out of PSUM; AV
  accumulation with M=65 (64 v-channels + a ones column giving the softmax
  denominator for free); the AV PSUM banks are drained by fast DVE copies so
  PE never waits on the normalization chain (reciprocal + DRAM-bounce
  partition-broadcast + multiply), which happens off the critical path; the
  output projection for i-tile t is emitted during i-tile t+1.

ScalarE (exp: 16.8M elements/core) is the roofline engine; sims are emitted
one slab ahead of AV so PE (in-order) never blocks on ScalarE.
"""
import os
import sys
import numpy as np

for _p in ("/opt/trn_rl_repo", "/root/.axon_site/_ro/trn_rl_repo"):
    if os.path.isdir(_p) and _p not in sys.path:
        sys.path.insert(0, _p)

import concourse.bass as bass
import concourse.mybir as mybir
from concourse.tile import TileContext
from concourse.bass_utils import run_bass_kernel_spmd

F32 = mybir.dt.float32
F32R = mybir.dt.float32r
EXP = mybir.ActivationFunctionType.Exp

B, H, D = 4, 8, 64
EQ, EK = 256, 512          # x channels, ctx channels
NQ, NK = 2048, 1024        # per-core query positions, kv positions
OC = 256                   # output channels
SCALE = D ** -0.5
IT, JT = NQ // 512, NK // 128   # 4 i-tiles of 512, 8 j-tiles of 128


def _split_excess_waits(nc, max_waits=1):
    """This walrus build rejects instructions carrying >max_waits sem waits;
    move the extras onto standalone nops just before (same engine, in-order,
    so semantics are unchanged)."""
    n_new = 0
    for f in nc.m.functions:
        for bb in f.blocks:
            insts = list(bb.instructions)
            out = []
            changed = False
            for inst in insts:
                si = inst.sync_info
                if si is not None and si.on_wait and len(si.on_wait) > max_waits:
                    waits = list(si.on_wait)
                    for w in waits[:-max_waits]:
                        nop = mybir.InstNoOp(
                            name=f"I-splitw-{n_new}",
                            sync_info=mybir.SyncInfo(on_wait=[w], on_update=[]),
                        )
                        nop.engine = inst.engine
                        n_new += 1
                        out.append(nop)
                        nc.register_instruction(nop, overwrite=True)
                    si.on_wait = waits[-max_waits:]
                    inst.sync_info = si
                    changed = True
                out.append(inst)
            if changed:
                bb.instructions.clear()
                bb.instructions.extend(out)
    return n_new


def _build():
    nc = bass.Bass()
    x_s = nc.declare_dram_parameter("x_s", [EQ, NQ], F32R, isOutput=False)
    ctx_s = nc.declare_dram_parameter("ctx_s", [EK, NK], F32R, isOutput=False)
    WqT = nc.declare_dram_parameter("WqT", [EQ, 512], F32R, isOutput=False)
    WkT = nc.declare_dram_parameter("WkT", [EK, 512], F32R, isOutput=False)
    WvT = nc.declare_dram_parameter("WvT", [EK, 512], F32R, isOutput=False)
    WoT = nc.declare_dram_parameter("WoT", [512, OC], F32R, isOutput=False)
    bo = nc.declare_dram_parameter("bo", [OC], F32, isOutput=False)
    y = nc.declare_dram_parameter("y", [OC, NQ], F32, isOutput=True)

    sscratch = nc.dram_tensor("sscratch", [IT, 4, 2, 512], F32)

    with TileContext(nc) as tc:
        with (
            tc.tile_pool(name="consts", bufs=1) as cp,
            tc.tile_pool(name="qkv", bufs=1) as qp,
            tc.tile_pool(name="exps", bufs=3) as ep,
            tc.tile_pool(name="avrp", bufs=4) as avrp,
            tc.tile_pool(name="avnp", bufs=8) as avnp,
            tc.tile_pool(name="work", bufs=2) as wp,
            tc.tile_pool(name="slab", bufs=2, space="PSUM") as slabp,
            tc.tile_pool(name="avp", bufs=1, space="PSUM") as avp,
            tc.tile_pool(name="yp", bufs=2, space="PSUM") as yp,
        ):
            # ---- const / weight / input loads (K,V deps first) ----
            wkt = cp.tile([128, 4 * 512], F32R, tag="wkt")
            wvt = cp.tile([128, 4 * 512], F32R, tag="wvt")
            ctx_sb = cp.tile([128, 4 * NK], F32R, tag="ctx_sb")
            wqt = cp.tile([128, 2 * 512], F32R, tag="wqt")
            x_sb = cp.tile([128, 2 * NQ], F32R, tag="x_sb")
            wot = cp.tile([128, 4 * OC], F32R, tag="wot")
            bo_col = cp.tile([128, 2], F32, tag="bo_col")
            for ec in range(4):
                nc.sync.dma_start(out=ctx_sb[:, ec * NK:(ec + 1) * NK],
                                  in_=ctx_s[ec * 128:(ec + 1) * 128, :])
                nc.sync.dma_start(out=wkt[:, ec * 512:(ec + 1) * 512],
                                  in_=WkT[ec * 128:(ec + 1) * 128, :])
                nc.sync.dma_start(out=wvt[:, ec * 512:(ec + 1) * 512],
                                  in_=WvT[ec * 128:(ec + 1) * 128, :])
            for ec in range(2):
                nc.sync.dma_start(out=wqt[:, ec * 512:(ec + 1) * 512],
                                  in_=WqT[ec * 128:(ec + 1) * 128, :])
                nc.sync.dma_start(out=x_sb[:, ec * NQ:(ec + 1) * NQ],
                                  in_=x_s[ec * 128:(ec + 1) * 128, :])
            for ec in range(4):
                nc.sync.dma_start(out=wot[:, ec * OC:(ec + 1) * OC],
                                  in_=WoT[ec * 128:(ec + 1) * 128, :])
            for ob in range(2):
                nc.sync.dma_start(out=bo_col[:, ob:ob + 1],
                                  in_=bo[ob * 128:(ob + 1) * 128])

            # persistent activations
            q_sb = qp.tile([128, 4 * NQ], F32R, tag="q_sb")      # [hp, i]
            k_sb = qp.tile([128, 4 * NK], F32R, tag="k_sb")      # [hp, j]
            vt_sb = qp.tile([128, JT * 520], F32R, tag="vt_sb")  # [jt, h*65 + c]

            # ones columns of vt (col 64 of each 65-block)
            vt_4d = vt_sb.rearrange("p (j h c) -> p j h c", j=JT, h=H)
            ones_f32 = cp.tile([128, JT * H], F32, tag="ones_f32")
            nc.vector.memset(ones_f32, 1.0)
            nc.vector.tensor_copy(
                vt_4d[:, :, :, 64:65],
                ones_f32.rearrange("p (j h) -> p j h", j=JT).unsqueeze(-1))

            # ---- K / VT projections (PSUM via yp pool; no extra banks) ----
            for hp in range(4):
                for ntile in range(NK // 512):
                    pk = yp.tile([128, 512], F32, tag="yps")
                    for ec in range(4):
                        nc.tensor.matmul(
                            pk,
                            lhsT=wkt[:, ec * 512 + hp * 128: ec * 512 + (hp + 1) * 128],
                            rhs=ctx_sb[:, ec * NK + ntile * 512: ec * NK + (ntile + 1) * 512],
                            start=(ec == 0), stop=(ec == 3))
                    nc.vector.tensor_copy(
                        k_sb[:, hp * NK + ntile * 512: hp * NK + (ntile + 1) * 512], pk)
            for jt in range(JT):
                pv = yp.tile([128, 512], F32, tag="yps")
                for ec in range(4):
                    nc.tensor.matmul(
                        pv,
                        lhsT=ctx_sb[:, ec * NK + jt * 128: ec * NK + (jt + 1) * 128],
                        rhs=wvt[:, ec * 512:(ec + 1) * 512],
                        start=(ec == 0), stop=(ec == 3))
                vt_t = vt_sb[:, jt * 520:(jt + 1) * 520].rearrange(
                    "p (h c) -> p h c", h=H)[:, :, 0:64]
                nc.vector.tensor_copy(vt_t, pv.rearrange("p (h c) -> p h c", c=64))

            def q_proj(hp):
                for ntile in range(IT):
                    pq = yp.tile([128, 512], F32, tag="yps")
                    for ec in range(2):
                        nc.tensor.matmul(
                            pq,
                            lhsT=wqt[:, ec * 512 + hp * 128: ec * 512 + (hp + 1) * 128],
                            rhs=x_sb[:, ec * NQ + ntile * 512: ec * NQ + (ntile + 1) * 512],
                            start=(ec == 0), stop=(ec == 1))
                    nc.vector.tensor_copy(
                        q_sb[:, hp * NQ + ntile * 512: hp * NQ + (ntile + 1) * 512], pq)

            q_proj(0)

            def sim_emit(hp, it, jt):
                slab = slabp.tile([128, 1024], F32, tag="slab")
                ks = slice(hp * NK + jt * 128, hp * NK + (jt + 1) * 128)
                qs = slice(hp * NQ + it * 512, hp * NQ + (it + 1) * 512)
                nc.tensor.matmul(
                    slab[:, 0:512], lhsT=k_sb[0:64, ks], rhs=q_sb[0:64, qs],
                    start=True, stop=True, tile_position=(0, 0))
                nc.tensor.matmul(
                    slab[:, 512:1024], lhsT=k_sb[64:128, ks], rhs=q_sb[64:128, qs],
                    start=True, stop=True, tile_position=(64, 0))
                return slab

            def oproj_emit(avn_tiles, it):
                for ob in range(2):
                    yps = yp.tile([128, 512], F32, tag="yps")
                    for cc in range(4):
                        nc.tensor.matmul(
                            yps,
                            lhsT=wot[:, cc * OC + ob * 128: cc * OC + (ob + 1) * 128],
                            rhs=avn_tiles[cc],
                            start=(cc == 0), stop=(cc == 3))
                    ysb = wp.tile([128, 512], F32, tag="ysb")
                    nc.vector.tensor_scalar_add(ysb, yps, bo_col[:, ob:ob + 1])
                    nc.sync.dma_start(
                        out=y[ob * 128:(ob + 1) * 128, it * 512:(it + 1) * 512],
                        in_=ysb)

            # ---- attention ----
            prev = None   # (avn_tiles, it) pending output projection
            for it in range(IT):
                pend = []  # per-hp (avr0, avr1, sbc) awaiting normalization
                for hp in range(4):
                    av0 = avp.tile([65, 512], F32, tag="av0")
                    av1 = avp.tile([65, 512], F32, tag="av1")
                    slab = sim_emit(hp, it, 0)
                    for jt in range(JT):
                        nslab = sim_emit(hp, it, jt + 1) if jt + 1 < JT else None
                        exps = ep.tile([128, 1024], F32R, tag="exps")
                        nc.scalar.activation(exps, slab, EXP, bias=0.0, scale=SCALE)
                        nc.tensor.matmul(
                            av0,
                            lhsT=vt_sb[:, jt * 520 + (2 * hp) * 65: jt * 520 + (2 * hp) * 65 + 65],
                            rhs=exps[:, 0:512],
                            start=(jt == 0), stop=(jt == JT - 1))
                        nc.tensor.matmul(
                            av1,
                            lhsT=vt_sb[:, jt * 520 + (2 * hp + 1) * 65: jt * 520 + (2 * hp + 1) * 65 + 65],
                            rhs=exps[:, 512:1024],
                            start=(jt == 0), stop=(jt == JT - 1))
                        slab = nslab
                    # drain AV PSUM banks quickly (frees them for hp+1)
                    avr = avrp.tile([128, 512], F32, tag="avr")
                    nc.vector.tensor_copy(avr[0:64, :], av0[0:64, :])
                    nc.vector.tensor_copy(avr[64:128, :], av1[0:64, :])
                    # softmax denominator -> reciprocal -> partition broadcast
                    srow_a = wp.tile([1, 512], F32, tag="srow_a")
                    srow_b = wp.tile([1, 512], F32, tag="srow_b")
                    nc.vector.reciprocal(srow_a, av0[64:65, :])
                    nc.vector.reciprocal(srow_b, av1[64:65, :])
                    nc.sync.dma_start(out=sscratch[it, hp, 0], in_=srow_a)
                    nc.sync.dma_start(out=sscratch[it, hp, 1], in_=srow_b)
                    sbc = wp.tile([128, 512], F32, tag="sbc")
                    base = ((it * 4 + hp) * 2) * 512
                    nc.sync.dma_start(
                        out=sbc[0:64, :],
                        in_=bass.AP(tensor=sscratch, offset=base,
                                    ap=[[0, 64], [1, 512]]))
                    nc.sync.dma_start(
                        out=sbc[64:128, :],
                        in_=bass.AP(tensor=sscratch, offset=base + 512,
                                    ap=[[0, 64], [1, 512]]))
                    pend.append((avr, sbc))
                    if hp == 0:
                        if it == 0:
                            for nhp in range(1, 4):
                                q_proj(nhp)
                        if prev is not None:
                            oproj_emit(*prev)
                            prev = None
                # normalization (off the PE critical path)
                avn_tiles = []
                for avr, sbc in pend:
                    avn = avnp.tile([128, 512], F32R, tag="avn")
                    nc.vector.tensor_mul(avn[0:64, :], avr[0:64, :], sbc[0:64, :])
                    nc.vector.tensor_mul(avn[64:128, :], avr[64:128, :], sbc[64:128, :])
                    avn_tiles.append(avn)
                prev = (avn_tiles, it)
            oproj_emit(*prev)

    _split_excess_waits(nc)
    return nc


_CACHED = None


def kernel(x, context, Wq, Wk, Wv, Wo, bo):
    global _CACHED
    if _CACHED is None:
        _CACHED = _build()
    nc = _CACHED

    x = np.asarray(x, dtype=np.float32)
    context = np.asarray(context, dtype=np.float32)
    xf = x.reshape(B, EQ, 64 * 64)
    cf = context.reshape(B, EK, 32 * 32)
    WqT = np.ascontiguousarray(np.asarray(Wq, np.float32).T)
    WkT = np.ascontiguousarray(np.asarray(Wk, np.float32).T)
    WvT = np.ascontiguousarray(np.asarray(Wv, np.float32).T)
    WoT = np.ascontiguousarray(np.asarray(Wo, np.float32).T)
    bo = np.ascontiguousarray(np.asarray(bo, np.float32))

    in_maps = []
    for core in range(8):
        b, half = core // 2, core % 2
        in_maps.append({
            "x_s": np.ascontiguousarray(xf[b, :, half * NQ:(half + 1) * NQ]),
            "ctx_s": np.ascontiguousarray(cf[b]),
            "WqT": WqT, "WkT": WkT, "WvT": WvT, "WoT": WoT, "bo": bo,
        })

    res = run_bass_kernel_spmd(nc, in_maps, list(range(8)))
    kernel.last_results = res

    out = np.empty((B, OC, 64 * 64), dtype=np.float32)
    for core in range(8):
        b, half = core // 2, core % 2
        out[b, :, half * NQ:(half + 1) * NQ] = res.results[core]["y"]
    return out.reshape(B, OC, 64, 64)


# revision 13
# speedup vs baseline: 1.6308x; 1.4464x over previous
"""CrossAttention Trainium2 kernel (8 NeuronCores, SPMD).

Problem: x [4,256,64,64], context [4,512,32,32], 8 heads x 64 dim,
q = Wq@x, k = Wk@ctx, v = Wv@ctx, attn = softmax(q^T k / 8), out = Wo@(v attn^T) + bo.

Sharding: fully data-parallel over (batch, query-spatial-half) -> 8 shards.
Each core computes K/V for its batch (duplicated per pair) and attention +
output projection for its 2048 query positions. Zero collectives.

Per-core pipeline (matmuls in float32r = full-rate):
  simT[j,i] = k^T q per head-pair, two K=64 matmuls packed onto PE row
  groups (tile_position); exp on ScalarE straight out of PSUM; AV
  accumulation with M=65 (64 v-channels + a ones column giving the softmax
  denominator for free); the AV PSUM banks are drained by fast DVE copies so
  PE never waits on the normalization chain (reciprocal + DRAM-bounce
  partition-broadcast + multiply), which happens off the critical path; the
  output projection for i-tile t is emitted during i-tile t+1.

ScalarE (exp: 16.8M elements/core) is the roofline engine; sims are emitted
one slab ahead of AV so PE (in-order) never blocks on ScalarE.
"""
import os
import sys
import numpy as np

for _p in ("/opt/trn_rl_repo", "/root/.axon_site/_ro/trn_rl_repo"):
    if os.path.isdir(_p) and _p not in sys.path:
        sys.path.insert(0, _p)

import concourse.bass as bass
import concourse.mybir as mybir
from concourse.tile import TileContext
from concourse.bass_utils import run_bass_kernel_spmd

F32 = mybir.dt.float32
F32R = mybir.dt.float32r
EXP = mybir.ActivationFunctionType.Exp

B, H, D = 4, 8, 64
EQ, EK = 256, 512          # x channels, ctx channels
NQ, NK = 2048, 1024        # per-core query positions, kv positions
OC = 256                   # output channels
SCALE = D ** -0.5
IT, JT = NQ // 512, NK // 128   # 4 i-tiles of 512, 8 j-tiles of 128


def _split_excess_waits(nc, max_waits=1):
    """This walrus build rejects instructions carrying >max_waits sem waits;
    move the extras onto standalone nops just before (same engine, in-order,
    so semantics are unchanged)."""
    n_new = 0
    for f in nc.m.functions:
        for bb in f.blocks:
            insts = list(bb.instructions)
            out = []
            changed = False
            for inst in insts:
                si = inst.sync_info
                if si is not None and si.on_wait and len(si.on_wait) > max_waits:
                    waits = list(si.on_wait)
                    for w in waits[:-max_waits]:
                        nop = mybir.InstNoOp(
                            name=f"I-splitw-{n_new}",
                            sync_info=mybir.SyncInfo(on_wait=[w], on_update=[]),
                        )
                        nop.engine = inst.engine
                        n_new += 1
                        out.append(nop)
                        nc.register_instruction(nop, overwrite=True)
                    si.on_wait = waits[-max_waits:]
                    inst.sync_info = si
                    changed = True
                out.append(inst)
            if changed:
                bb.instructions.clear()
                bb.instructions.extend(out)
    return n_new


def _build():
    nc = bass.Bass()
    x_s = nc.declare_dram_parameter("x_s", [EQ, NQ], F32R, isOutput=False)
    ctx_s = nc.declare_dram_parameter("ctx_s", [EK, NK], F32R, isOutput=False)
    WqT = nc.declare_dram_parameter("WqT", [EQ, 512], F32R, isOutput=False)
    WkT = nc.declare_dram_parameter("WkT", [EK, 512], F32R, isOutput=False)
    WvT = nc.declare_dram_parameter("WvT", [EK, 512], F32R, isOutput=False)
    WoT = nc.declare_dram_parameter("WoT", [512, OC], F32R, isOutput=False)
    bo = nc.declare_dram_parameter("bo", [OC], F32, isOutput=False)
    y = nc.declare_dram_parameter("y", [OC, NQ], F32, isOutput=True)

    sscratch = nc.dram_tensor("sscratch", [IT, 4, 2, 512], F32)
    sscratch2 = nc.dram_tensor("sscratch2", [IT, 4096], F32)

    with TileContext(nc) as tc:
        with (
            tc.tile_pool(name="consts", bufs=1) as cp,
            tc.tile_pool(name="qkv", bufs=1) as qp,
            tc.tile_pool(name="exps", bufs=4) as ep,
            tc.tile_pool(name="avrp", bufs=4) as avrp,
            tc.tile_pool(name="avnp", bufs=8) as avnp,
            tc.tile_pool(name="work", bufs=3) as wp,
            tc.tile_pool(name="slab", bufs=2, space="PSUM") as slabp,
            tc.tile_pool(name="avp", bufs=1, space="PSUM") as avp,
            tc.tile_pool(name="yp", bufs=2, space="PSUM") as yp,
        ):
            # ---- const / weight / input loads (K,V deps first) ----
            wkt = cp.tile([128, 4 * 512], F32R, tag="wkt")
            wvt = cp.tile([128, 4 * 512], F32R, tag="wvt")
            ctx_sb = cp.tile([128, 4 * NK], F32R, tag="ctx_sb")
            wqt = cp.tile([128, 2 * 512], F32R, tag="wqt")
            x_sb = cp.tile([128, 2 * NQ], F32R, tag="x_sb")
            wot = cp.tile([128, 4 * OC], F32R, tag="wot")
            bo_col = cp.tile([128, 2], F32, tag="bo_col")
            for ec in range(4):
                nc.sync.dma_start(out=ctx_sb[:, ec * NK:(ec + 1) * NK],
                                  in_=ctx_s[ec * 128:(ec + 1) * 128, :])
                nc.sync.dma_start(out=wkt[:, ec * 512:(ec + 1) * 512],
                                  in_=WkT[ec * 128:(ec + 1) * 128, :])
                nc.sync.dma_start(out=wvt[:, ec * 512:(ec + 1) * 512],
                                  in_=WvT[ec * 128:(ec + 1) * 128, :])
            for ec in range(2):
                nc.sync.dma_start(out=wqt[:, ec * 512:(ec + 1) * 512],
                                  in_=WqT[ec * 128:(ec + 1) * 128, :])
                nc.sync.dma_start(out=x_sb[:, ec * NQ:(ec + 1) * NQ],
                                  in_=x_s[ec * 128:(ec + 1) * 128, :])
            for ec in range(4):
                nc.sync.dma_start(out=wot[:, ec * OC:(ec + 1) * OC],
                                  in_=WoT[ec * 128:(ec + 1) * 128, :])
            for ob in range(2):
                nc.sync.dma_start(out=bo_col[:, ob:ob + 1],
                                  in_=bo[ob * 128:(ob + 1) * 128])

            # persistent activations
            q_sb = qp.tile([128, 4 * NQ], F32R, tag="q_sb")      # [hp, i]
            k_sb = qp.tile([128, 4 * NK], F32R, tag="k_sb")      # [hp, j]
            vt_sb = qp.tile([128, JT * 520], F32R, tag="vt_sb")  # [jt, h*65 + c]

            # ones columns of vt (col 64 of each 65-block)
            vt_4d = vt_sb.rearrange("p (j h c) -> p j h c", j=JT, h=H)
            ones_f32 = cp.tile([128, JT * H], F32, tag="ones_f32")
            nc.vector.memset(ones_f32, 1.0)
            nc.vector.tensor_copy(
                vt_4d[:, :, :, 64:65],
                ones_f32.rearrange("p (j h) -> p j h", j=JT).unsqueeze(-1))

            # ---- K / VT projections (PSUM via yp pool; no extra banks) ----
            for hp in range(4):
                for ntile in range(NK // 512):
                    pk = yp.tile([128, 512], F32, tag="yps")
                    for ec in range(4):
                        nc.tensor.matmul(
                            pk,
                            lhsT=wkt[:, ec * 512 + hp * 128: ec * 512 + (hp + 1) * 128],
                            rhs=ctx_sb[:, ec * NK + ntile * 512: ec * NK + (ntile + 1) * 512],
                            start=(ec == 0), stop=(ec == 3))
                    nc.vector.tensor_copy(
                        k_sb[:, hp * NK + ntile * 512: hp * NK + (ntile + 1) * 512], pk)
            for jt in range(JT):
                pv = yp.tile([128, 512], F32, tag="yps")
                for ec in range(4):
                    nc.tensor.matmul(
                        pv,
                        lhsT=ctx_sb[:, ec * NK + jt * 128: ec * NK + (jt + 1) * 128],
                        rhs=wvt[:, ec * 512:(ec + 1) * 512],
                        start=(ec == 0), stop=(ec == 3))
                vt_t = vt_sb[:, jt * 520:(jt + 1) * 520].rearrange(
                    "p (h c) -> p h c", h=H)[:, :, 0:64]
                nc.vector.tensor_copy(vt_t, pv.rearrange("p (h c) -> p h c", c=64))

            def q_proj(hp):
                for ntile in range(IT):
                    pq = yp.tile([128, 512], F32, tag="yps")
                    for ec in range(2):
                        nc.tensor.matmul(
                            pq,
                            lhsT=wqt[:, ec * 512 + hp * 128: ec * 512 + (hp + 1) * 128],
                            rhs=x_sb[:, ec * NQ + ntile * 512: ec * NQ + (ntile + 1) * 512],
                            start=(ec == 0), stop=(ec == 1))
                    nc.vector.tensor_copy(
                        q_sb[:, hp * NQ + ntile * 512: hp * NQ + (ntile + 1) * 512], pq)

            q_proj(0)

            def sim_emit(hp, it, jt):
                slab = slabp.tile([128, 1024], F32, tag="slab")
                ks = slice(hp * NK + jt * 128, hp * NK + (jt + 1) * 128)
                qs = slice(hp * NQ + it * 512, hp * NQ + (it + 1) * 512)
                nc.tensor.matmul(
                    slab[:, 0:512], lhsT=k_sb[0:64, ks], rhs=q_sb[0:64, qs],
                    start=True, stop=True, tile_position=(0, 0))
                nc.tensor.matmul(
                    slab[:, 512:1024], lhsT=k_sb[64:128, ks], rhs=q_sb[64:128, qs],
                    start=True, stop=True, tile_position=(64, 0))
                return slab

            def oproj_emit(avn_tiles, it):
                for ob in range(2):
                    yps = yp.tile([128, 512], F32, tag="yps")
                    for cc in range(4):
                        nc.tensor.matmul(
                            yps,
                            lhsT=wot[:, cc * OC + ob * 128: cc * OC + (ob + 1) * 128],
                            rhs=avn_tiles[cc],
                            start=(cc == 0), stop=(cc == 3))
                    ysb = wp.tile([128, 512], F32, tag="ysb")
                    nc.vector.tensor_scalar_add(ysb, yps, bo_col[:, ob:ob + 1])
                    nc.sync.dma_start(
                        out=y[ob * 128:(ob + 1) * 128, it * 512:(it + 1) * 512],
                        in_=ysb)

            # ---- attention ----
            prev = None   # (avn_tiles, it) pending output projection
            for it in range(IT):
                pend = []  # per-hp (avr0, avr1, sbc) awaiting normalization
                for hp in range(4):
                    av0 = avp.tile([65, 512], F32, tag="av0")
                    av1 = avp.tile([65, 512], F32, tag="av1")
                    slab = sim_emit(hp, it, 0)
                    for jt in range(JT):
                        nslab = sim_emit(hp, it, jt + 1) if jt + 1 < JT else None
                        exps = ep.tile([128, 1024], F32R, tag="exps")
                        nc.scalar.activation(exps, slab, EXP, bias=0.0, scale=SCALE)
                        nc.tensor.matmul(
                            av0,
                            lhsT=vt_sb[:, jt * 520 + (2 * hp) * 65: jt * 520 + (2 * hp) * 65 + 65],
                            rhs=exps[:, 0:512],
                            start=(jt == 0), stop=(jt == JT - 1))
                        nc.tensor.matmul(
                            av1,
                            lhsT=vt_sb[:, jt * 520 + (2 * hp + 1) * 65: jt * 520 + (2 * hp + 1) * 65 + 65],
                            rhs=exps[:, 512:1024],
                            start=(jt == 0), stop=(jt == JT - 1))
                        slab = nslab
                    # drain AV PSUM banks quickly (frees them for hp+1);
                    # row 64 carries the softmax denominator
                    avr0 = avrp.tile([65, 512], F32, tag="avr0")
                    avr1 = avrp.tile([65, 512], F32, tag="avr1")
                    nc.vector.tensor_copy(avr0, av0)
                    nc.vector.tensor_copy(avr1, av1)
                    nc.sync.dma_start(out=sscratch[it, hp, 0], in_=avr0[64:65, :])
                    nc.sync.dma_start(out=sscratch[it, hp, 1], in_=avr1[64:65, :])
                    pend.append((avr0, avr1))
                    if hp == 0:
                        if it == 0:
                            for nhp in range(1, 4):
                                q_proj(nhp)
                        if prev is not None:
                            oproj_emit(*prev)
                            prev = None
                # normalization (off the PE critical path):
                # one lane-parallel reciprocal over all 8 denominator rows
                stile = wp.tile([128, 32], F32, tag="stile")
                nc.sync.dma_start(
                    out=stile,
                    in_=bass.AP(tensor=sscratch, offset=it * 4096,
                                ap=[[32, 128], [1, 32]]))
                stile_r = wp.tile([128, 32], F32, tag="stile_r")
                nc.vector.reciprocal(stile_r, stile)
                nc.sync.dma_start(
                    out=sscratch2[it].rearrange("(p f) -> p f", p=128),
                    in_=stile_r)
                avn_tiles = []
                for hp, (avr0, avr1) in enumerate(pend):
                    base = it * 4096 + hp * 1024
                    sbc_a = wp.tile([64, 512], F32, tag="sbc_a")
                    sbc_b = wp.tile([64, 512], F32, tag="sbc_b")
                    nc.sync.dma_start(
                        out=sbc_a,
                        in_=bass.AP(tensor=sscratch2, offset=base,
                                    ap=[[0, 64], [1, 512]]))
                    nc.sync.dma_start(
                        out=sbc_b,
                        in_=bass.AP(tensor=sscratch2, offset=base + 512,
                                    ap=[[0, 64], [1, 512]]))
                    avn = avnp.tile([128, 512], F32R, tag="avn")
                    nc.vector.tensor_mul(avn[0:64, :], avr0[0:64, :], sbc_a)
                    nc.vector.tensor_mul(avn[64:128, :], avr1[0:64, :], sbc_b)
                    avn_tiles.append(avn)
                prev = (avn_tiles, it)
            oproj_emit(*prev)

    _split_excess_waits(nc)
    return nc


_CACHED = None


def kernel(x, context, Wq, Wk, Wv, Wo, bo):
    global _CACHED
    if _CACHED is None:
        _CACHED = _build()
    nc = _CACHED

    x = np.asarray(x, dtype=np.float32)
    context = np.asarray(context, dtype=np.float32)
    xf = x.reshape(B, EQ, 64 * 64)
    cf = context.reshape(B, EK, 32 * 32)
    WqT = np.ascontiguousarray(np.asarray(Wq, np.float32).T)
    WkT = np.ascontiguousarray(np.asarray(Wk, np.float32).T)
    WvT = np.ascontiguousarray(np.asarray(Wv, np.float32).T)
    WoT = np.ascontiguousarray(np.asarray(Wo, np.float32).T)
    bo = np.ascontiguousarray(np.asarray(bo, np.float32))

    in_maps = []
    for core in range(8):
        b, half = core // 2, core % 2
        in_maps.append({
            "x_s": np.ascontiguousarray(xf[b, :, half * NQ:(half + 1) * NQ]),
            "ctx_s": np.ascontiguousarray(cf[b]),
            "WqT": WqT, "WkT": WkT, "WvT": WvT, "WoT": WoT, "bo": bo,
        })

    res = run_bass_kernel_spmd(nc, in_maps, list(range(8)))
    kernel.last_results = res

    out = np.empty((B, OC, 64 * 64), dtype=np.float32)
    for core in range(8):
        b, half = core // 2, core % 2
        out[b, :, half * NQ:(half + 1) * NQ] = res.results[core]["y"]
    return out.reshape(B, OC, 64, 64)
